# revision 14
# baseline (speedup 1.0000x reference)
"""8-core Trainium2 Bass kernel for a 2-layer GCN + mean-pool + 4-layer MLP.

Strategy (graph/data parallel, per the sharding hint):
  - Nodes are partitioned into 8 contiguous ranges of 6250 (core c owns
    [c*6250, (c+1)*6250)).  Edges are bucketed twice on the host: by
    dst-owner (aggregation work) and by src-owner (out-degree counting).
  - Within a core, edges are sorted by their local dst (resp. src) and laid
    out on a uniform [49 windows x T tiles x 128 slots] grid so the SPMD
    program is identical on every core; empty slots carry src=0 / rel=-1.
  - Aggregation per 128-edge tile is a one-hot "selection matrix" matmul
    accumulated in PSUM over each 128-node window; a ones-column in the rhs
    simultaneously produces in-degrees.  Out-degrees come from a counting
    pass of the same shape.  Normalization (D^-1/2 A D^-1/2) is applied by
    scaling table rows with src_isqrt before the gather and scaling the
    PSUM output with dst_isqrt.
  - The (h @ W) * src_isqrt "message tables" are built shard-wise and
    replicated with an AllGather; per-edge rows are fetched from the table
    with indirect-DMA gathers (128 rows x 256B per descriptor).
  - Per-graph pooled sums+counts [64,129] are AllReduce'd, and the small MLP
    runs replicated on every core.
"""

import os
import sys

import numpy as np

sys.path.insert(0, "/opt/trn_rl_repo")

import ml_dtypes

BF16 = ml_dtypes.bfloat16

N = 50000
E = 1600000
D = 128
G = 64
C = 8
NS = N // C            # 6250 nodes per core
P = 128
NT = (NS + P - 1) // P  # 49 windows / node tiles per core
NSP = NT * P            # 6272


# ---------------------------------------------------------------------------
# Host-side sharding prep
# ---------------------------------------------------------------------------

HALF = 25088


def _chunks(n):
    return [8] * (n // 8) + ([n % 8] if n % 8 else [])


def _wrap_idx(vals):
    """vals [sz*128] int16 -> [128, sz*8] wrapped (idx k at (k%16, k//16)),
    replicated across the eight 16-partition stripes."""
    s = len(vals) // 16
    blk = vals.reshape(s, 16).T
    return np.tile(blk, (8, 1))


def _edge_grid_split(dst_local, src_global, TL, TH):
    """Per-window [lo-src tiles | hi-src tiles] grid.

    Returns (esw int16 [P, NT*(TL+TH)*8] wrapped gather indices,
             edst_rel bf16 [P, NT*(TL+TH)])."""
    T = TL + TH
    half = (src_global >= HALF).astype(np.int64)
    key = dst_local // P * 2 + half
    order = np.argsort(key, kind="stable")
    key_s = key[order]
    src_s = src_global[order]
    rel_s = (dst_local - (dst_local // P) * P)[order]
    esw = np.zeros((P, NT * T * 8), dtype=np.int16)
    edst_rel = np.full((P, NT * T), -1.0, dtype=BF16)
    for wi in range(NT):
        for seg, (tbase, tlen, base_row) in enumerate(
                [(0, TL, 0), (TL, TH, HALF)]):
            s = int(np.searchsorted(key_s, 2 * wi + seg))
            e = int(np.searchsorted(key_s, 2 * wi + seg, side="right"))
            cnt = e - s
            assert cnt <= tlen * P, f"segment overflow {cnt} > {tlen * P}"
            j = np.arange(cnt)
            edst_rel[j % P, wi * T + tbase + j // P] = rel_s[s:e].astype(BF16)
            vals = np.zeros(tlen * P, dtype=np.int16)
            vals[j] = (src_s[s:e] - base_row).astype(np.int16)
            b = 0
            for sz in _chunks(tlen):
                col0 = (wi * T + tbase + b) * 8
                esw[:, col0:col0 + sz * 8] = _wrap_idx(
                    vals[b * P:(b + sz) * P])
                b += sz
    return esw, edst_rel


def _edge_grid(key_local, other, T):
    """Sort edges by local key, lay out on [P, NT*T] grid.

    Returns (other_grid int32, rel_grid bf16) where slot (p, w*T + t) holds
    edge w-window's (t*128+p)-th edge; empty slots: other=0, rel=-1.
    """
    order = np.argsort(key_local, kind="stable")
    key_s = key_local[order]
    oth_s = other[order]
    w = key_s // P
    other_grid = np.zeros((P, NT * T), dtype=np.int32)
    rel_grid = np.full((P, NT * T), -1.0, dtype=BF16)
    starts = np.searchsorted(w, np.arange(NT))
    ends = np.searchsorted(w, np.arange(NT), side="right")
    for wi in range(NT):
        s, e = int(starts[wi]), int(ends[wi])
        cnt = e - s
        assert cnt <= T * P, f"window overflow {cnt} > {T * P}"
        j = np.arange(cnt)
        t_idx = j // P
        p_idx = j % P
        other_grid[p_idx, wi * T + t_idx] = oth_s[s:e]
        rel_grid[p_idx, wi * T + t_idx] = (key_s[s:e] - wi * P).astype(BF16)
    return other_grid, rel_grid


def _prep_shards(x, src, dst, graph_id):
    src = np.asarray(src).astype(np.int64)
    dst = np.asarray(dst).astype(np.int64)
    x = np.asarray(x).astype(np.float32)
    graph_id = np.asarray(graph_id).astype(np.int64)

    dst_owner = dst // NS
    src_owner = src // NS
    TL = 0
    TH = 0
    Tc = 0
    masks = []
    for c in range(C):
        me = dst_owner == c
        ms = src_owner == c
        wloc = (dst[me] - c * NS) // P
        lo = src[me] < HALF
        cnt_lo = np.bincount(wloc[lo], minlength=NT)
        cnt_hi = np.bincount(wloc[~lo], minlength=NT)
        cnt_s = np.bincount((src[ms] - c * NS) // P, minlength=NT)
        TL = max(TL, int(np.ceil(cnt_lo.max() / P)))
        TH = max(TH, int(np.ceil(cnt_hi.max() / P)))
        Tc = max(Tc, int(np.ceil(cnt_s.max() / P)))
        masks.append((me, ms))
    T = TL + TH

    shards = []
    for c in range(C):
        me, ms = masks[c]
        esrc, edst_rel = _edge_grid_split(dst[me] - c * NS, src[me], TL, TH)
        csrc_loc = src[ms] - c * NS
        _, csrc_rel = _edge_grid(csrc_loc, csrc_loc, Tc)
        xT = np.zeros((P, NSP), dtype=BF16)
        xT[:, :NS] = x[c * NS:(c + 1) * NS].T.astype(BF16)
        gid = np.full((P, NT), -1.0, dtype=BF16)
        gid.T.flat[:NS] = graph_id[c * NS:(c + 1) * NS].astype(BF16)
        shards.append(dict(esrc=esrc, edst=edst_rel, csrc=csrc_rel,
                           xT=xT, gid=gid))
    return shards, TL, TH, Tc


# ---------------------------------------------------------------------------
# Bass program
# ---------------------------------------------------------------------------

_PROGRAM_CACHE = {}


def _build_program(TL, TH, Tc):
    T = TL + TH
    _skip = {""}  # diagnostic phase-skip hooks; always run the full program
    _skip_gather = "gather" in _skip
    import concourse.bacc as bacc
    import concourse.bass as bass
    import concourse.mybir as mybir
    import concourse.tile as tile

    f32 = mybir.dt.float32
    bf16 = mybir.dt.bfloat16
    i32 = mybir.dt.int32
    Alu = mybir.AluOpType
    Act = mybir.ActivationFunctionType

    nc = bacc.Bacc("TRN2", target_bir_lowering=False, debug=False,
                   num_devices=C)

    # ---- kernel I/O ----
    t_esrc = nc.dram_tensor("esrc", [P, NT * T * 8], mybir.dt.int16, kind="ExternalInput")
    t_edst = nc.dram_tensor("edst", [P, NT * T], bf16, kind="ExternalInput")
    t_csrc = nc.dram_tensor("csrc", [P, NT * Tc], bf16, kind="ExternalInput")
    t_xT = nc.dram_tensor("xT", [P, NSP], bf16, kind="ExternalInput")
    t_gid = nc.dram_tensor("gid", [P, NT], bf16, kind="ExternalInput")
    t_iota = nc.dram_tensor("iota", [P, P], bf16, kind="ExternalInput")
    t_ident = nc.dram_tensor("ident", [P, P], bf16, kind="ExternalInput")
    t_id64 = nc.dram_tensor("id64", [G, G], f32, kind="ExternalInput")
    t_W1 = nc.dram_tensor("W1", [D, D], bf16, kind="ExternalInput")
    t_W2 = nc.dram_tensor("W2", [D, D], bf16, kind="ExternalInput")
    t_b1r = nc.dram_tensor("b1r", [P, D], f32, kind="ExternalInput")
    t_b2r = nc.dram_tensor("b2r", [P, D], f32, kind="ExternalInput")
    t_Wc1 = nc.dram_tensor("Wc1", [D, 64], f32, kind="ExternalInput")
    t_Wc2 = nc.dram_tensor("Wc2", [64, 32], f32, kind="ExternalInput")
    t_Wc3 = nc.dram_tensor("Wc3", [32, 16], f32, kind="ExternalInput")
    t_Wc4 = nc.dram_tensor("Wc4", [16, 1], f32, kind="ExternalInput")
    t_bc1 = nc.dram_tensor("bc1", [64, 1], f32, kind="ExternalInput")
    t_bc2 = nc.dram_tensor("bc2", [32, 1], f32, kind="ExternalInput")
    t_bc3 = nc.dram_tensor("bc3", [16, 1], f32, kind="ExternalInput")
    t_bc4 = nc.dram_tensor("bc4", [1, 1], f32, kind="ExternalInput")
    t_out = nc.dram_tensor("out", [1, G], f32, kind="ExternalOutput")

    rg = [list(range(C))]

    with tile.TileContext(nc) as tc:
        with (
            tc.tile_pool(name="const", bufs=1) as cp,
            tc.tile_pool(name="dram", bufs=1, space="DRAM") as dp,
            tc.tile_pool(name="sgen", bufs=4) as sp,
            tc.tile_pool(name="tmp", bufs=6) as tp,
            tc.tile_pool(name="msg", bufs=1) as mp,
        ):
            # ---- persistent SBUF tensors ----
            esrc_sb = cp.tile([P, NT * T * 8], mybir.dt.int16)
            edst_sb = cp.tile([P, NT * T], bf16)
            csrc_sb = cp.tile([P, NT * Tc], bf16)
            xT_sb = cp.tile([P, NSP], bf16)
            gid_sb = cp.tile([P, NT], bf16)
            iota_sb = cp.tile([P, P], bf16)
            ident_sb = cp.tile([P, P], bf16)
            id64_sb = cp.tile([G, G], f32)
            W1_sb = cp.tile([D, D], bf16)
            W2_sb = cp.tile([D, D], bf16)
            b1r_sb = cp.tile([P, D], f32)
            b2r_sb = cp.tile([P, D], f32)
            Wc1_sb = cp.tile([D, 64], f32)
            Wc2_sb = cp.tile([64, 32], f32)
            Wc3_sb = cp.tile([32, 16], f32)
            Wc4_sb = cp.tile([16, 1], f32)
            bc1_sb = cp.tile([64, 1], f32)
            bc2_sb = cp.tile([32, 1], f32)
            bc3_sb = cp.tile([16, 1], f32)
            bc4_sb = cp.tile([1, 1], f32)
            ones_sb = cp.tile([P, 1], bf16)
            s_isq_sb = cp.tile([P, NT], f32)
            d_isq_sb = cp.tile([P, NT], f32)
            h1_sb = cp.tile([P, NSP], bf16)
            h1T_sb = cp.tile([P, NSP], bf16)
            h2e_sb = cp.tile([P, NT * 129], bf16)
            msg0_sb = cp.tile([P, T * 128], bf16)
            msg1_sb = cp.tile([P, T * 128], bf16)
            msg2_sb = cp.tile([P, T * 128], bf16)
            msg3_sb = cp.tile([P, T * 128], bf16)
            msg4_sb = cp.tile([P, T * 128], bf16)
            msg5_sb = cp.tile([P, T * 128], bf16)
            msgs = [msg0_sb, msg1_sb, msg2_sb, msg3_sb, msg4_sb, msg5_sb]

            for dst_sb, src_t in [
                (esrc_sb, t_esrc), (edst_sb, t_edst), (csrc_sb, t_csrc),
                (xT_sb, t_xT), (gid_sb, t_gid), (iota_sb, t_iota),
                (ident_sb, t_ident), (id64_sb, t_id64), (W1_sb, t_W1),
                (W2_sb, t_W2), (b1r_sb, t_b1r), (b2r_sb, t_b2r),
                (Wc1_sb, t_Wc1), (Wc2_sb, t_Wc2), (Wc3_sb, t_Wc3),
                (Wc4_sb, t_Wc4), (bc1_sb, t_bc1), (bc2_sb, t_bc2),
                (bc3_sb, t_bc3), (bc4_sb, t_bc4),
            ]:
                nc.sync.dma_start(out=dst_sb[:], in_=src_t[:])
            nc.vector.memset(ones_sb[:], 1.0)
            nc.vector.memset(h2e_sb[:], 1.0)
            for m in msgs:
                nc.vector.memset(m[:], 1.0)

            # ---- DRAM intermediates ----
            shard1 = dp.tile([NS, D], bf16)
            table1 = dp.tile([N, D], bf16, addr_space="Shared")
            shard2 = dp.tile([NS, D], bf16)
            table2 = dp.tile([N, D], bf16, addr_space="Shared")
            ar_in = dp.tile([G, 129], f32)
            ar_out = dp.tile([G, 129], f32, addr_space="Shared")

            # ================= Phase A: out-degree counting ================
            if "count" in _skip:
                nc.vector.memset(s_isq_sb[:], 1.0)
            with tc.tile_pool(name="psA", bufs=4, space="PSUM") as psA:
                for w in range(NT if "count" not in _skip else 0):
                    ps = psA.tile([P, 1], f32)
                    for t in range(Tc):
                        col = w * Tc + t
                        Sc = sp.tile([P, P], bf16, tag="Sc")
                        nc.vector.tensor_tensor(
                            out=Sc[:], in0=iota_sb[:],
                            in1=csrc_sb[:, col:col + 1].to_broadcast([P, P]),
                            op=Alu.is_equal)
                        nc.tensor.matmul(ps[:], lhsT=Sc[:], rhs=ones_sb[:],
                                         start=(t == 0), stop=(t == Tc - 1))
                    cnt_t = tp.tile([P, 1], f32, tag="cnt")
                    nc.vector.tensor_scalar(out=cnt_t[:], in0=ps[:],
                                            scalar1=1.0, scalar2=None,
                                            op0=Alu.max)
                    nc.scalar.activation(s_isq_sb[:, w:w + 1], cnt_t[:],
                                         Act.Abs_reciprocal_sqrt)

            # ============ Phase A2: in-degree counting (dst grid) ==========
            with tc.tile_pool(name="psA2", bufs=4, space="PSUM") as psA2:
                for w in range(NT):
                    ps = psA2.tile([P, 1], f32)
                    for t in range(T):
                        col = w * T + t
                        Sc = sp.tile([P, P], bf16, tag="Sc2")
                        nc.vector.tensor_tensor(
                            out=Sc[:], in0=iota_sb[:],
                            in1=edst_sb[:, col:col + 1].to_broadcast([P, P]),
                            op=Alu.is_equal)
                        nc.tensor.matmul(ps[:], lhsT=Sc[:], rhs=ones_sb[:],
                                         start=(t == 0), stop=(t == T - 1))
                    cnt_t = tp.tile([P, 1], f32, tag="cnt2")
                    nc.vector.tensor_scalar(out=cnt_t[:], in0=ps[:],
                                            scalar1=1.0, scalar2=None,
                                            op0=Alu.max)
                    nc.scalar.activation(d_isq_sb[:, w:w + 1], cnt_t[:],
                                         Act.Abs_reciprocal_sqrt)

            # ================= helper: table build + allgather =============
            def build_table(hT_src_sb, W_sb, shard, table):
                with tc.tile_pool(name="psB", bufs=4, space="PSUM") as psB:
                    for i in range(NT):
                        ps = psB.tile([P, D], f32)
                        nc.tensor.matmul(
                            ps[:], lhsT=hT_src_sb[:, i * P:(i + 1) * P],
                            rhs=W_sb[:], start=True, stop=True)
                        sc_t = tp.tile([P, D], bf16, tag="sct")
                        nc.vector.tensor_scalar(
                            out=sc_t[:], in0=ps[:],
                            scalar1=s_isq_sb[:, i:i + 1], scalar2=None,
                            op0=Alu.mult)
                        lo = i * P
                        hi = min((i + 1) * P, NS)
                        if hi > lo:
                            nc.sync.dma_start(out=shard[lo:hi, :],
                                              in_=sc_t[:hi - lo, :])
                if "ag" not in _skip:
                    nc.gpsimd.collective_compute(
                        "AllGather", Alu.bypass, replica_groups=rg,
                        ins=[shard.opt()], outs=[table.opt()])
                else:
                    nc.sync.dma_start(out=table[0:NS, :], in_=shard[:])

            # ================= helper: conv layer ==========================
            def conv_layer(table, brd_sb, out_cb):
                """out_cb(w, pre_relu_tile) consumes window output."""
                with tc.tile_pool(name="psC", bufs=6, space="PSUM") as psC:
                    for w in range(NT):
                        mbuf = msgs[w % 6]
                        gview = mbuf[:].rearrange("p (t c) -> p t c", c=128)
                        for tbase, tlen, r0, r1 in [(0, TL, 0, HALF),
                                                    (TL, TH, HALF, N)]:
                            b = 0
                            for sz in _chunks(tlen):
                                babs = tbase + b
                                col0 = (w * T + babs) * 8
                                nc.gpsimd.dma_gather(
                                    out_ap=gview[:, babs:babs + sz, :],
                                    in_ap=table[r0:r1, :],
                                    idxs_ap=esrc_sb[:, col0:col0 + sz * 8],
                                    num_idxs=sz * 128,
                                    num_idxs_reg=sz * 128,
                                    elem_size=128,
                                    single_packet=False,
                                )
                                b += sz
                        ps = psC.tile([P, D], f32)
                        for t in range(T):
                            col = w * T + t
                            S = sp.tile([P, P], bf16, tag="S")
                            nc.vector.tensor_tensor(
                                out=S[:], in0=iota_sb[:],
                                in1=edst_sb[:, col:col + 1].to_broadcast([P, P]),
                                op=Alu.is_equal)
                            nc.tensor.matmul(
                                ps[:], lhsT=S[:],
                                rhs=mbuf[:, t * 128:(t + 1) * 128],
                                start=(t == 0), stop=(t == T - 1))
                        pre_t = tp.tile([P, D], f32, tag="pre")
                        nc.vector.scalar_tensor_tensor(
                            out=pre_t[:], in0=ps[:, 0:D],
                            scalar=d_isq_sb[:, w:w + 1], in1=brd_sb[:],
                            op0=Alu.mult, op1=Alu.add)
                        out_cb(w, pre_t)

            # ================= Layer 1 =====================================
            build_table(xT_sb, W1_sb, shard1, table1)

            def l1_out(w, pre_t):
                nc.vector.tensor_scalar(
                    out=h1_sb[:, w * P:(w + 1) * P], in0=pre_t[:],
                    scalar1=0.0, scalar2=None, op0=Alu.max)

            conv_layer(table1, b1r_sb, l1_out)

            # transpose h1 tiles -> h1T
            with tc.tile_pool(name="psT", bufs=4, space="PSUM") as psT:
                for i in range(NT):
                    pst = psT.tile([P, P], bf16)
                    nc.tensor.transpose(pst[:], h1_sb[:, i * P:(i + 1) * P],
                                        ident_sb[:])
                    nc.vector.tensor_copy(h1T_sb[:, i * P:(i + 1) * P],
                                          pst[:])

            # ================= Layer 2 =====================================
            build_table(h1T_sb, W2_sb, shard2, table2)

            def l2_out(w, pre_t):
                nc.vector.tensor_scalar(
                    out=h2e_sb[:, w * 129:w * 129 + D], in0=pre_t[:],
                    scalar1=0.0, scalar2=None, op0=Alu.max)

            conv_layer(table2, b2r_sb, l2_out)

            # ================= Pooling + AllReduce =========================
            with tc.tile_pool(name="psP", bufs=2, space="PSUM") as psP:
                psp = psP.tile([G, 129], f32)
                for i in range(NT):
                    Sp = sp.tile([P, G], bf16, tag="Sp")
                    nc.vector.tensor_tensor(
                        out=Sp[:], in0=iota_sb[:, :G],
                        in1=gid_sb[:, i:i + 1].to_broadcast([P, G]),
                        op=Alu.is_equal)
                    nc.tensor.matmul(psp[:], lhsT=Sp[:],
                                     rhs=h2e_sb[:, i * 129:(i + 1) * 129],
                                     start=(i == 0), stop=(i == NT - 1))
                pool_sb = tp.tile([G, 129], f32, tag="pool")
                nc.vector.tensor_copy(pool_sb[:], psp[:])
                nc.sync.dma_start(out=ar_in[:], in_=pool_sb[:])

            nc.gpsimd.collective_compute(
                "AllReduce", Alu.add, replica_groups=rg,
                ins=[ar_in.opt()], outs=[ar_out.opt()])

            # ================= mean + MLP ==================================
            with tc.tile_pool(name="psM", bufs=1, space="PSUM") as psM:
                red_sb = tp.tile([G, 129], f32, tag="red")
                nc.sync.dma_start(out=red_sb[:], in_=ar_out[:])
                pcnt = tp.tile([G, 1], f32, tag="pcnt")
                nc.vector.tensor_scalar(out=pcnt[:], in0=red_sb[:, D:D + 1],
                                        scalar1=1.0, scalar2=None, op0=Alu.max)
                prcp = tp.tile([G, 1], f32, tag="prcp")
                nc.vector.reciprocal(prcp[:], pcnt[:])
                hg_sb = tp.tile([G, D], f32, tag="hg")
                nc.vector.tensor_scalar(out=hg_sb[:], in0=red_sb[:, 0:D],
                                        scalar1=prcp[:, :1], scalar2=None,
                                        op0=Alu.mult)
                ps_hgT = psM.tile([D, G], f32)
                nc.tensor.transpose(ps_hgT[:], hg_sb[:], id64_sb[:])
                hgT_sb = tp.tile([D, G], f32, tag="hgT")
                nc.vector.tensor_copy(hgT_sb[:], ps_hgT[:])

                ps1 = psM.tile([64, G], f32)
                nc.tensor.matmul(ps1[:], lhsT=Wc1_sb[:], rhs=hgT_sb[:],
                                 start=True, stop=True)
                o1_sb = tp.tile([64, G], f32, tag="o1")
                nc.scalar.activation(o1_sb[:], ps1[:], Act.Relu,
                                     bias=bc1_sb[:, :1])
                ps2 = psM.tile([32, G], f32)
                nc.tensor.matmul(ps2[:], lhsT=Wc2_sb[:], rhs=o1_sb[:],
                                 start=True, stop=True)
                o2_sb = tp.tile([32, G], f32, tag="o2")
                nc.scalar.activation(o2_sb[:], ps2[:], Act.Relu,
                                     bias=bc2_sb[:, :1])
                ps3 = psM.tile([16, G], f32)
                nc.tensor.matmul(ps3[:], lhsT=Wc3_sb[:], rhs=o2_sb[:],
                                 start=True, stop=True)
                o3_sb = tp.tile([16, G], f32, tag="o3")
                nc.scalar.activation(o3_sb[:], ps3[:], Act.Relu,
                                     bias=bc3_sb[:, :1])
                ps4 = psM.tile([1, G], f32)
                nc.tensor.matmul(ps4[:], lhsT=Wc4_sb[:], rhs=o3_sb[:],
                                 start=True, stop=True)
                out_sb = tp.tile([1, G], f32, tag="osb")
                nc.vector.tensor_scalar(out=out_sb[:], in0=ps4[:],
                                        scalar1=bc4_sb[:1, :1], scalar2=None,
                                        op0=Alu.add)
                nc.sync.dma_start(out=t_out[:], in_=out_sb[:])

    nc.compile()
    return nc


# ---------------------------------------------------------------------------
# Entry point
# ---------------------------------------------------------------------------

def kernel(x, src, dst, graph_id, num_graphs, W1, b1, W2, b2,
           Wc1, bc1, Wc2, bc2, Wc3, bc3, Wc4, bc4):
    import concourse.bass_utils as bass_utils

    assert int(num_graphs) == G

    shards, TL, TH, Tc = _prep_shards(x, src, dst, graph_id)

    iota = np.broadcast_to(np.arange(P, dtype=np.float32), (P, P)).astype(BF16)
    ident = np.eye(P, dtype=BF16)
    id64 = np.eye(G, dtype=np.float32)
    W1b = np.asarray(W1).astype(BF16)
    W2b = np.asarray(W2).astype(BF16)
    b1r = np.broadcast_to(np.asarray(b1, dtype=np.float32), (P, D)).copy()
    b2r = np.broadcast_to(np.asarray(b2, dtype=np.float32), (P, D)).copy()

    common = dict(
        iota=iota, ident=ident, id64=id64, W1=W1b, W2=W2b, b1r=b1r, b2r=b2r,
        Wc1=np.asarray(Wc1, dtype=np.float32),
        Wc2=np.asarray(Wc2, dtype=np.float32),
        Wc3=np.asarray(Wc3, dtype=np.float32),
        Wc4=np.asarray(Wc4, dtype=np.float32),
        bc1=np.asarray(bc1, dtype=np.float32).reshape(64, 1),
        bc2=np.asarray(bc2, dtype=np.float32).reshape(32, 1),
        bc3=np.asarray(bc3, dtype=np.float32).reshape(16, 1),
        bc4=np.asarray(bc4, dtype=np.float32).reshape(1, 1),
    )

    in_maps = []
    for c in range(C):
        sh = shards[c]
        in_maps.append(dict(
            esrc=sh["esrc"], edst=sh["edst"], csrc=sh["csrc"],
            xT=sh["xT"], gid=sh["gid"], **common))

    key = (TL, TH, Tc)
    if key not in _PROGRAM_CACHE:
        _PROGRAM_CACHE[key] = _build_program(TL, TH, Tc)
    nc = _PROGRAM_CACHE[key]

    global _last_in_maps
    _last_in_maps = in_maps

    res = bass_utils.run_bass_kernel_spmd(nc, in_maps, core_ids=list(range(C)))
    out = res.results[0]["out"]
    return np.asarray(out, dtype=np.float32).reshape(G, 1)


if __name__ == "__main__":
    import jax
    with jax.default_device(jax.devices("cpu")[0]):
        import reference
        inputs = reference.setup_inputs()
        inp = {k: (np.asarray(v) if hasattr(v, "shape") else v)
               for k, v in inputs.items()}
        expected = np.asarray(reference.reference(**inputs))
    got = kernel(**inp)
    err = np.abs(got - expected).max()
    rel = err / (np.abs(expected).max() + 1e-12)
    print("absmax err:", err, "rel:", rel)
